# revision 38
# baseline (speedup 1.0000x reference)
"""Trainium2 Bass kernel for nn_CauseAttention (dense transformer block).

Reference computation (per batch b):
    qkv = x @ W_attn + b_attn          # [T, 3N]
    q, k, v  (heads H=16, HD=64)
    att = softmax(q k^T / sqrt(T))     # NOTE scale 1/sqrt(T) = 1/32, no mask
    y   = att @ v                      # [T, N]
    out = y @ W_proj + b_proj

Sharding: data-parallel over batch. B=16 batches across 8 NeuronCores,
2 batches per core. Each core gets its x slice + full weights and computes
its output slice. No collectives.

Per-core kernel layout strategy (everything bf16 on the matmul path,
fp32 PSUM accumulation; validated rel-err ~3e-3 vs fp32 reference):

  - x^T (features on partitions) produced once per batch via a bf16 DRAM
    scratch + HWDGE xbar DMA transpose.
  - qk^T = (x @ W_qk)^T computed directly in transposed layout:
    lhsT = W_attn chunk [128k, 128n], rhs = x^T -> psum [n, tok].
  - v computed in natural layout [tok, feat]: lhsT = x^T tok-slice,
    rhs = W_v. Stored in 65-wide per-head blocks [v(64) | ones] so that the
    second attention matmul also produces softmax row-sums for free
    (y' rows 0..63, sums in row 64).
  - S^T[ki,qi] = k^T.T @ q^T per 128-row ki tile (contraction d=64; even
    heads use PE rows 0-63, odd heads rows 64-127 which lets the HW overlap
    head pairs in different row-groups).
  - exp on ScalarE straight from PSUM with scale=1/32 (logits are in
    [-0.8, 0.8], so softmax max-subtraction is unnecessary), output bf16.
  - y'^T (+ sums) = [v|ones].T @ att^T accumulated over ki tiles.
  - normalization: recip(s) on DVE (fast approx), broadcast across 64
    partitions via a rank-1 PE matmul, multiply into y^T (bf16).
  - out = y^T.T @ W_proj + b_proj  (lhsT = y^T which we already have).

The harness calls kernel(**inputs) with FULL inputs; we shard internally
and run one SPMD NEFF on cores 0..7 via the axon/PJRT path.
"""

import sys

for _p in ("/opt/trn_rl_repo", "/opt/pypackages"):
    if _p not in sys.path:
        sys.path.append(_p)

import numpy as np

import concourse.bass as bass
import concourse.mybir as mybir
import concourse.tile as tile
from concourse import bacc
from concourse.masks import make_identity

F32 = mybir.dt.float32
BF16 = mybir.dt.bfloat16
AF = mybir.ActivationFunctionType
ALU = mybir.AluOpType

# Problem shapes (hardcoded per spec)
B, T, N, H = 16, 1024, 1024, 16
HD = N // H              # 64
SCALE = 1.0 / np.sqrt(T)  # = 1/32
NCORES = 8
BL = B // NCORES         # batches per core = 2
P = 128
KT = N // P              # 8 k-tiles of the model dim
NF = (2 * N) // P        # 16 qk output tiles
NT = T // P              # 8 token tiles per batch
CH = 2                   # 512-wide chunks per 1024 free dim
C512 = 512


def _build_nc() -> bass.Bass:
    nc = bacc.Bacc("TRN2", target_bir_lowering=False, debug=False, num_devices=NCORES)

    x = nc.dram_tensor("x", [BL * T, N], F32, kind="ExternalInput").ap()
    wa = nc.dram_tensor("W_attn", [N, 3 * N], F32, kind="ExternalInput").ap()
    ba = nc.dram_tensor("b_attn", [3 * N], F32, kind="ExternalInput").ap()
    wp = nc.dram_tensor("W_proj", [N, N], F32, kind="ExternalInput").ap()
    bp = nc.dram_tensor("b_proj", [N], F32, kind="ExternalInput").ap()
    out = nc.dram_tensor("out", [BL * T, N], F32, kind="ExternalOutput").ap()

    with tile.TileContext(nc) as tc:
        with (
            tc.tile_pool(name="wpool", bufs=1) as wpool,
            tc.tile_pool(name="bpool", bufs=1) as bpool,
            tc.tile_pool(name="apool", bufs=2) as apool,
            tc.tile_pool(name="npool", bufs=1) as npool,
            tc.tile_pool(name="opool", bufs=3) as opool,
            tc.tile_pool(name="ps", bufs=2, space="PSUM") as ps_pool,
            tc.tile_pool(name="psy", bufs=2, space="PSUM") as psy_pool,
        ):
            # ---------------- x cast + batch-0 transpose first ----------------
            # Emission order == scheduler priority: get batch-0's x pipeline
            # onto the DMA queues before the big weight casts so PE starts
            # transposing within a few us.
            identity = wpool.tile([P, P], BF16, name="identity")
            make_identity(nc, identity)

            def emit_xT(b):
                # per-tt SWDGE cast loads (f32 DRAM -> bf16 SBUF); multi-wait
                # legalization is handled by Bacc.generate_event_semaphores
                x_sb = bpool.tile([P, NT, N], BF16, name="x_sb", tag="x_sb")
                x_r = x[b * T : (b + 1) * T, :].rearrange("(tt p) n -> p tt n", p=P)
                for tt in range(NT):
                    nc.gpsimd.dma_start(x_sb[:, tt, :], x_r[:, tt, :])
                xT = bpool.tile([P, KT, T], BF16, name="xT", tag="xT")
                for tt in range(NT):
                    for kf in range(KT):
                        pst = ps_pool.tile([P, P], BF16, tag="mm", name="ps_tr")
                        nc.tensor.transpose(
                            pst[:], x_sb[:, tt, kf * P : (kf + 1) * P], identity[:]
                        )
                        nc.vector.tensor_copy(xT[:, kf, tt * P : (tt + 1) * P], pst[:])
                return xT

            xT0 = emit_xT(0)

            # ---------------- weights / constants (once) ----------------
            # chunked so dependent matmuls can start early (q cols first)
            wa_sb = wpool.tile([P, KT, 3 * N], BF16, name="wa_sb")
            wa_r = wa.rearrange("(kt p) n -> p kt n", p=P)
            for c0 in range(0, 3 * N, N):
                nc.gpsimd.dma_start(wa_sb[:, :, c0 : c0 + N], wa_r[:, :, c0 : c0 + N])
            bqk_sb = wpool.tile([P, NF], F32, name="bqk_sb")
            nc.sync.dma_start(bqk_sb[:], ba[0 : 2 * N].rearrange("(o p) -> p o", p=P))
            ones_row = wpool.tile([P, P], BF16, name="ones_row")
            nc.vector.memset(ones_row[:], 1.0)

            # bias rows (bf16) -> broadcast across partitions via rank-1 matmul
            bv_row = apool.tile([1, N], BF16, name="bv_row", tag="attT", bufs=6)
            nc.gpsimd.dma_start(bv_row[:], ba[2 * N : 3 * N].rearrange("(a n) -> a n", a=1))
            bp_row = apool.tile([1, N], BF16, name="bp_row", tag="attT", bufs=6)
            nc.gpsimd.dma_start(bp_row[:], bp.rearrange("(a n) -> a n", a=1))

            bv_bc = wpool.tile([P, N], BF16, name="bv_bc")
            bp_bc = wpool.tile([P, N], BF16, name="bp_bc")
            for row, bc in ((bv_row, bv_bc), (bp_row, bp_bc)):
                pst = psy_pool.tile([P, N], F32, tag="y", name="ps_bcast")
                for c in range(CH):
                    cs = slice(c * C512, (c + 1) * C512)
                    nc.tensor.matmul(pst[:, cs], ones_row[0:1, :], row[:, cs],
                                     start=True, stop=True)
                nc.vector.tensor_copy(bc[:], pst[:])

            # proj weights (needed latest)
            wp_sb = wpool.tile([P, KT, N], BF16, name="wp_sb")
            nc.gpsimd.dma_start(wp_sb[:], wp.rearrange("(kt p) n -> p kt n", p=P))

            for b in range(BL):
                xT = xT0 if b == 0 else emit_xT(b)

                # ---------------- qk^T = (x @ W_qk)^T + b ------------------
                qkT = bpool.tile([P, NF, T], BF16, name="qkT", tag="qkT")
                for nf in range(NF):
                    pst = ps_pool.tile([P, T], F32, tag="mm", name="ps_qk")
                    for kt in range(KT):
                        for c in range(CH):
                            cs = slice(c * C512, (c + 1) * C512)
                            nc.tensor.matmul(
                                pst[:, cs],
                                wa_sb[:, kt, nf * P : (nf + 1) * P],
                                xT[:, kt, cs],
                                start=(kt == 0),
                                stop=(kt == KT - 1),
                            )
                    nc.vector.tensor_scalar_add(
                        qkT[:, nf, :], pst[:], bqk_sb[:, nf : nf + 1]
                    )

                # ---------------- v = x @ W_v + b (natural layout) ----------
                # per-head 65-wide blocks [v(64) | ones] so that the second
                # attention matmul also emits softmax row-sums (row 64)
                v_sb = bpool.tile([P, NT, H, 65], BF16, name="v_sb", tag="v_sb")
                nc.vector.memset(v_sb[:, :, :, 64:65], 1.0)
                for tt in range(NT):
                    pst = ps_pool.tile([P, N], F32, tag="mm", name="ps_v")
                    for kt in range(KT):
                        for c in range(CH):
                            cs = slice(c * C512, (c + 1) * C512)
                            nc.tensor.matmul(
                                pst[:, cs],
                                xT[:, kt, tt * P : (tt + 1) * P],
                                wa_sb[:, kt, 2 * N + c * C512 : 2 * N + (c + 1) * C512],
                                start=(kt == 0),
                                stop=(kt == KT - 1),
                            )
                    nc.vector.tensor_tensor(
                        v_sb[:, tt, :, 0:64],
                        pst.rearrange("p (h d) -> p h d", d=HD),
                        bv_bc.rearrange("p (h d) -> p h d", d=HD),
                        ALU.add,
                    )

                # ---------------- attention ----------------
                yT = bpool.tile([P, KT, T], BF16, name="yT", tag="yT", bufs=2)
                if b == 0:
                    _DBG_TILES.update(xT=xT, qkT=qkT, v_sb=v_sb, yT=yT)

                def emit_norm(h, psy):
                    # y' rows 0..63, sums row 64 (all heads).  Odd heads'
                    # normalized y must land at partitions 64..127 of yT for
                    # the projection lhsT; DVE can't shift partitions, so odd
                    # heads go through a small SBUF->SBUF DMA.
                    rf = npool.tile([P, T], F32, name="recip_f", tag="recip_f")
                    # NOTE: reciprocal_approx_fast (custom DVE op) miscomputes on
                    # partition-sliced APs on HW; run it over the full tile (same
                    # cost — DVE time scales with free dim only) and use row 64.
                    nc.vector.reciprocal_approx_fast(rf[:, :], psy[:, :])
                    # rb slots: row 64 = bf16 recip (bcast rhs); rows 0..63
                    # reused as the odd-head y staging (disjoint subtiles).
                    rb = npool.tile([P, T], BF16, name="recip_b", tag="recip_b", bufs=2)
                    nc.vector.tensor_copy(rb[64:65, :], rf[64:65, :])
                    psb = ps_pool.tile([P, T], F32, tag="mm", name="ps_bc")
                    for c in range(CH):
                        cs = slice(c * C512, (c + 1) * C512)
                        nc.tensor.matmul(
                            psb[0:64, cs],
                            ones_row[64:65, 0:64],
                            rb[64:65, cs],
                            start=True,
                            stop=True,
                        )
                    bcs = npool.tile([P, T], BF16, name="bc_sb", tag="bc_sb")
                    nc.vector.tensor_copy(bcs[0:64, :], psb[0:64, :])
                    if h % 2 == 0:
                        nc.vector.tensor_tensor(
                            yT[0:64, h // 2, :], psy[0:64, :], bcs[0:64, :], ALU.mult
                        )
                    else:
                        ytmp = npool.tile([P, T], BF16, name="ytmp", tag="recip_b", bufs=2)
                        nc.vector.tensor_tensor(
                            ytmp[0:64, :], psy[0:64, :], bcs[0:64, :], ALU.mult
                        )
                        nc.sync.dma_start(yT[64:128, h // 2, :], ytmp[0:64, :])

                pending = None  # (h, psy) for one-head-deferred normalization
                for h in range(H):
                    par = h % 2
                    base = par * 64
                    qT_h = qkT[base : base + 64, h // 2, :]
                    kT_h = qkT[base : base + 64, 8 + h // 2, :]

                    att_kt = []
                    for kt in range(KT):
                        pst = ps_pool.tile([P, T], F32, tag="mm", name="ps_s")
                        for c in range(CH):
                            cs = slice(c * C512, (c + 1) * C512)
                            nc.tensor.matmul(
                                pst[:, cs],
                                kT_h[:, kt * P : (kt + 1) * P],
                                qT_h[:, cs],
                                start=True,
                                stop=True,
                            )
                        attT = apool.tile([P, T], BF16, name="attT", tag="attT", bufs=6)
                        nc.scalar.activation(
                            attT[:], pst[:], AF.Exp, scale=float(SCALE)
                        )
                        att_kt.append(attT)
                        if b == 0 and h == 0:
                            _DBG_TILES[f"attT_h0_k{kt}"] = attT

                    psy = psy_pool.tile([P, T], F32, tag="y", name="ps_y")
                    for kt in range(KT):
                        for c in range(CH):
                            cs = slice(c * C512, (c + 1) * C512)
                            nc.tensor.matmul(
                                psy[0:65, cs],
                                v_sb[:, kt, h, 0:65],
                                att_kt[kt][:, cs],
                                start=(kt == 0),
                                stop=(kt == KT - 1),
                            )
                    if pending is not None:
                        emit_norm(*pending)
                    pending = (h, psy)
                emit_norm(*pending)

                # ---------------- output projection ----------------
                for tt in range(NT):
                    pst = ps_pool.tile([P, N], F32, tag="mm", name="ps_o")
                    for kt in range(KT):
                        for c in range(CH):
                            cs = slice(c * C512, (c + 1) * C512)
                            nc.tensor.matmul(
                                pst[:, cs],
                                yT[:, kt, tt * P : (tt + 1) * P],
                                wp_sb[:, kt, cs],
                                start=(kt == 0),
                                stop=(kt == KT - 1),
                            )
                    for c in range(CH):
                        cs = slice(c * C512, (c + 1) * C512)
                        osb = opool.tile([P, C512], F32, name="osb", tag="osb", bufs=2)
                        nc.vector.tensor_tensor(
                            osb[:], pst[:, cs], bp_bc[:, cs], ALU.add
                        )
                        nc.sync.dma_start(
                            out[b * T + tt * P : b * T + (tt + 1) * P, cs], osb[:]
                        )

    nc.compile()
    return nc


_CACHE: dict = {}
_DBG_TILES: dict = {}  # name -> AP, populated during _build_nc for sim debugging


def _get_runner():
    """Build the Bass module once and wrap it in a cached jitted PJRT callable
    (mirrors concourse.bass2jax.run_bass_via_pjrt, but with a stable jit so
    repeated kernel() calls don't recompile)."""
    if "runner" in _CACHE:
        return _CACHE["runner"]

    import jax
    from jax.experimental.shard_map import shard_map
    from jax.sharding import Mesh, PartitionSpec

    from concourse import bass2jax

    nc = _build_nc()
    bass2jax.install_neuronx_cc_hook()

    partition_name = (
        nc.partition_id_tensor.name if nc.partition_id_tensor is not None else None
    )
    in_names: list[str] = []
    out_names: list[str] = []
    out_avals = []
    zero_outs: list[np.ndarray] = []
    for alloc in nc.m.functions[0].allocations:
        if not isinstance(alloc, mybir.MemoryLocationSet):
            continue
        name = alloc.memorylocations[0].name
        if alloc.kind == "ExternalInput":
            if name != partition_name:
                in_names.append(name)
        elif alloc.kind == "ExternalOutput":
            shape = tuple(alloc.tensor_shape)
            dtype = mybir.dt.np(alloc.dtype)
            out_names.append(name)
            out_avals.append(jax.core.ShapedArray(shape, dtype))
            zero_outs.append(np.zeros(shape, dtype))

    n_params = len(in_names)
    all_names = list(in_names) + list(out_names)
    if partition_name is not None:
        all_names.append(partition_name)
    all_names = tuple(all_names)
    donate = tuple(range(n_params, n_params + len(out_names)))

    def _body(*args):
        operands = list(args)
        if partition_name is not None:
            operands.append(bass2jax.partition_id_tensor())
        outs = bass2jax._bass_exec_p.bind(
            *operands,
            out_avals=tuple(out_avals),
            in_names=all_names,
            out_names=tuple(out_names),
            lowering_input_output_aliases=(),
            sim_require_finite=True,
            sim_require_nnan=True,
            nc=nc,
        )
        return tuple(outs)

    devices = jax.devices()[:NCORES]
    mesh = Mesh(np.asarray(devices), ("core",))
    n_io = n_params + len(out_names)
    sharded = jax.jit(
        shard_map(
            _body,
            mesh=mesh,
            in_specs=(PartitionSpec("core"),) * n_io,
            out_specs=(PartitionSpec("core"),) * len(out_names),
            check_rep=False,
        ),
        donate_argnums=donate,
        keep_unused=True,
    )

    def prep(per_core_inputs: list[dict]):
        """Stage concatenated inputs on the devices once; returns a thunk that
        launches one execution (fresh donated zero outputs each call)."""
        from jax.sharding import NamedSharding

        sh = NamedSharding(mesh, PartitionSpec("core"))
        concat_in = [
            jax.device_put(
                np.concatenate(
                    [np.asarray(m[name]) for m in per_core_inputs], axis=0
                ),
                sh,
            )
            for name in in_names
        ]
        mk_zeros = jax.jit(
            lambda: tuple(
                jax.numpy.zeros((NCORES * z.shape[0], *z.shape[1:]), z.dtype)
                for z in zero_outs
            ),
            out_shardings=(sh,) * len(zero_outs),
        )

        def launch():
            zs = mk_zeros()
            return sharded(*concat_in, *zs)

        return launch

    def run(per_core_inputs: list[dict]) -> list[np.ndarray]:
        """per_core_inputs: one dict per core keyed by in_names.
        Returns the per-core 'out' arrays."""
        out_arrs = prep(per_core_inputs)()
        (res,) = [np.asarray(a) for a in out_arrs]
        per_core_shape = out_avals[0].shape
        return list(res.reshape(NCORES, *per_core_shape))

    run.prep = prep
    _CACHE["runner"] = run
    return run


def _per_core_maps(inputs: dict) -> list[dict]:
    x = np.ascontiguousarray(np.asarray(inputs["x"], dtype=np.float32))
    wa = np.ascontiguousarray(np.asarray(inputs["W_attn"], dtype=np.float32))
    ba = np.ascontiguousarray(np.asarray(inputs["b_attn"], dtype=np.float32))
    wp = np.ascontiguousarray(np.asarray(inputs["W_proj"], dtype=np.float32))
    bp = np.ascontiguousarray(np.asarray(inputs["b_proj"], dtype=np.float32))
    x_slices = x.reshape(NCORES, BL * T, N)
    return [
        {"x": x_slices[i], "W_attn": wa, "b_attn": ba, "W_proj": wp, "b_proj": bp}
        for i in range(NCORES)
    ]


def kernel(**inputs) -> np.ndarray:
    run = _get_runner()
    outs = run(_per_core_maps(inputs))
    return np.concatenate(outs, axis=0).reshape(B, T, N).astype(np.float32)


if __name__ == "__main__":
    rng = np.random.default_rng(0)
    ins = {
        "x": rng.standard_normal((B, T, N), dtype=np.float32),
        "W_attn": (rng.standard_normal((N, 3 * N)) * 0.02).astype(np.float32),
        "b_attn": (rng.standard_normal((3 * N,)) * 0.02).astype(np.float32),
        "W_proj": (rng.standard_normal((N, N)) * 0.02).astype(np.float32),
        "b_proj": (rng.standard_normal((N,)) * 0.02).astype(np.float32),
    }
    out = kernel(**ins)
    print("kernel out:", out.shape, out.dtype, float(np.abs(out).mean()))
